# revision 87
# baseline (speedup 1.0000x reference)
"""Trainium2 Bass kernel for nn_CrossAttention (B=2, S=C=4096, D=512, H=8, Dh=64).

Sharding: batch x head-pair parallel over 8 cores. Core c handles batch
b = c//4 and heads {2*(c%4), 2*(c%4)+1}. Each core computes full attention
for its two heads plus its partial contribution to the output projection;
the host sums the 4 per-core partials per batch and adds the bias.

All-bf16 dataflow (inputs pre-rounded on host; fp32 PSUM accumulation):
  kT [128=2*dh, C] = wk_sb.T @ ctx_b          (N=512 moving)
  qT [128=2*dh, S] = wq_sb.T @ x_b
  v  [c, 128=2*dh] = ctx_b.T @ wv_sb          (N=128 moving, bf16)
  s chunk pair -> TWO PSUM tiles sa/sb [128c, 512q] = kT_h.T @ qT_h; each
      tile has exactly ONE exp reader (DVE does sa as Schraudolph bit-trick
      exp -- bf16_bits = int16(A*s + B) via int16 bitcast -- and ACT does sb
      as true exp; the slower DVE gets the earlier-finishing chunk so the
      per-group exp envelope is tightest).  One reader per tile keeps
      the tile framework from chaining the two exps behind each other, so
      the ACT and DVE software pipelines run fully decoupled and the
      s-pool WAR chain (bufs=2 each) is half a group shorter.
  o_aug [128q, 65] += P_chunk.T @ v_aug_chunk  (P stationary, v moving
      N=65; col 64 = ones -> softmax denominator lands per-q-partition;
      flush lags exp issue by LAG groups)
  o2n [128q, 512] = o/den via DVE reciprocal + ONE broadcast multiply
      (issued at the END of each qb so the next qb's first PV never waits)
  oT = one batched XBAR dma transpose (3D out => 4x 128x128 tiles)
  y   [128s, 512]  = oT.T @ woT (K=128, both heads at once) -> bf16 y
      partials (host accumulates in f32)

Engine budget per group (~745ns of PE matmul): ACT 612ns exp + the
projection/y copies, DVE 658ns exp + the normalize; all PSUM-reading work
must stay on ACT/DVE (GPSIMD cannot touch PSUM).  The final qb's epilogue
uses PE transposes via an identity matrix (no XBAR sem latency) with
copies alternating ACT/DVE.  Inputs load as ONE packed DMA per 512-col
block spanning all 4 contraction chunks (DMA cost is dominated by per-
instruction generation, not bytes), n-major so kproj/vproj pace with
arrival; the v ones-columns are memset (a strided DMA of ones costs
~3.6us of descriptor time).
"""

import numpy as np
import ml_dtypes
from contextlib import ExitStack

import concourse.bass as bass
import concourse.tile as tile
from concourse import bacc, mybir
from concourse.bass_utils import run_bass_kernel_spmd

F32 = mybir.dt.float32
BF16 = mybir.dt.bfloat16
I16 = mybir.dt.int16
EXP = mybir.ActivationFunctionType.Exp
MULT = mybir.AluOpType.mult
ADD = mybir.AluOpType.add

B = 2
S = 4096
C = 4096
D = 512
DH = 64
SCALE = DH ** -0.5  # 0.125
NKC = D // 128      # 4 contraction chunks
NQB = S // 512      # 8 query blocks
NCB = C // 128      # 32 context chunks of 128
NG = NCB // 2       # 16 chunk groups of 2 per (h, qb)
VW = DH + 1         # 65
VROW = NCB * VW     # per-head width of the v_aug tile

LAG = 3             # PV flush lag (groups); keeps PE off the o_ps WAR
QPROJ_G = 14         # issue qproj(qb+1) at (h=1, g=QPROJ_G)

# Schraudolph exp in bf16-bits domain: bits = int16(A*s + B)
SCH_A = SCALE * 128.0 / float(np.log(2.0))
SCH_B = 16256.0 - 5.25

_CACHE = {}


def build_nc():
    nc = bacc.Bacc("TRN2", target_bir_lowering=False, debug=False)

    xT = nc.dram_tensor("xT", [D, S], BF16, kind="ExternalInput").ap()
    ctxT = nc.dram_tensor("ctxT", [D, C], BF16, kind="ExternalInput").ap()
    wqT = nc.dram_tensor("wqT", [D, 128], BF16, kind="ExternalInput").ap()
    wkT = nc.dram_tensor("wkT", [D, 128], BF16, kind="ExternalInput").ap()
    wvT = nc.dram_tensor("wvT", [D, 128], BF16, kind="ExternalInput").ap()
    woT = nc.dram_tensor("woT", [128, D], BF16, kind="ExternalInput").ap()
    ident = nc.dram_tensor("ident", [128, 128], BF16,
                           kind="ExternalInput").ap()
    y = nc.dram_tensor("y", [S, D], BF16, kind="ExternalOutput").ap()

    def o_off(qs, h):
        # o_aug slice offsets inside the [128, 1024] accumulator; regions
        # are padded to a uniform 128-word stride (so the 8 denominators at
        # +64 form one strided AP) and stay inside their 512-word bank.
        return (qs // 2) * 512 + ((qs % 2) * 2 + h) * 128

    with tile.TileContext(nc) as tc, ExitStack() as ctx:
        sb = ctx.enter_context(tc.tile_pool(name="sb", bufs=1))

        # ---- persistent SBUF tiles ----
        wq_sb = sb.tile([128, D], BF16, name="wq_sb")
        wk_sb = sb.tile([128, D], BF16, name="wk_sb")
        wv_sb = sb.tile([128, D], BF16, name="wv_sb")
        wo_sb = sb.tile([128, D], BF16, name="wo_sb")
        kT_sb = sb.tile([128, C], BF16, name="kT_sb")
        qT_sb = sb.tile([128, S], BF16, name="qT_sb")
        v_sb = sb.tile([128, 2 * VROW], BF16, name="v_sb")
        ident_sb = sb.tile([128, 128], BF16, name="ident_sb")

        with tc.tile_pool(name="aps", bufs=1, space="PSUM") as aps, \
             tc.tile_pool(name="inbig", bufs=8) as inbig, \
             tc.tile_pool(name="psb", bufs=6) as psb, \
             tc.tile_pool(name="msb", bufs=2) as msb:
            ctx_b = [inbig.tile([128, NKC * 512], BF16, name=f"ctxb{n}",
                                tag="in", bufs=16) for n in range(NQB)]
            x_b = [inbig.tile([128, NKC * 512], BF16, name=f"xb{n}",
                              tag="in", bufs=16) for n in range(NQB)]

            # ---- DMA order tuned for ramp: k/q weights, block 0 of both
            # inputs, v/o weights, then the remaining blocks n-major so
            # kproj(n)/vproj(4n..4n+3) can start as block n lands.
            def dma_w(dst, src):
                nc.sync.dma_start(
                    dst.rearrange("p (kc m) -> p kc m", m=128),
                    src.rearrange("(kc p) m -> p kc m", p=128))

            def load_block(dst, srcT, n):
                # one DMA per 512-col block spanning all NKC row-chunks:
                # generation cost is per-DMA, so packed blocks keep the DMA
                # pipeline transfer-bound instead of generation-bound
                nc.sync.dma_start(
                    dst.rearrange("p (kc m) -> p kc m", m=512),
                    srcT.rearrange("(kc p) m -> p kc m",
                                   p=128)[:, :, n * 512:(n + 1) * 512])

            def load_half(dst, srcT, n, half):
                k0 = half * 2
                nc.sync.dma_start(
                    dst[:, k0 * 512:(k0 + 2) * 512].rearrange(
                        "p (kc m) -> p kc m", m=512),
                    srcT.rearrange("(kc p) m -> p kc m",
                                   p=128)[:, k0:k0 + 2,
                                          n * 512:(n + 1) * 512])

            dma_w(wk_sb, wkT)
            load_half(ctx_b[0], ctxT, 0, 0)
            dma_w(wq_sb, wqT)
            load_half(x_b[0], xT, 0, 0)
            load_half(ctx_b[0], ctxT, 0, 1)
            load_half(x_b[0], xT, 0, 1)
            dma_w(wv_sb, wvT)
            # ones columns of v_aug via memset (a strided DMA of ones costs
            # ~3.6us of descriptor time and blocks the ctx loads)
            v4 = v_sb.rearrange("p (h c k) -> p h c k", h=2, k=VW)
            nc.gpsimd.memset(v4[:, :, :, DH:VW], 1.0)
            for n in range(1, NQB):
                load_block(ctx_b[n], ctxT, n)
            nc.sync.dma_start(wo_sb[:], woT)
            nc.sync.dma_start(ident_sb[:], ident)
            for n in range(1, NQB):
                load_block(x_b[n], xT, n)

            # warm the PE p-state: the clock model needs ~3us of
            # continuous busy to reach 2.4GHz, and the first projections
            # otherwise run at 0.65-1.2GHz while waiting on input DMA
            dum_sb = sb.tile([128, 128], BF16, name="dum_sb")
            nc.gpsimd.memset(dum_sb[:], 0.0)
            dum_ps = aps.tile([128, 128], F32, name="dum_ps", tag="y",
                              bufs=2)
            for _ in range(16):
                nc.tensor.matmul(dum_ps[:], dum_sb[:], dum_sb[:],
                                 start=True, stop=True)

            def kproj(n):
                pk = aps.tile([128, 512], F32, name=f"pk{n}", tag="y", bufs=2)
                for kc in range(NKC):
                    nc.tensor.matmul(pk[:], wk_sb[:, kc * 128:(kc + 1) * 128],
                                     ctx_b[n][:, kc * 512:(kc + 1) * 512],
                                     start=(kc == 0), stop=(kc == NKC - 1))
                if n % 2 == 0:
                    nc.scalar.copy(kT_sb[:, n * 512:(n + 1) * 512], pk[:])
                else:
                    nc.vector.tensor_copy(kT_sb[:, n * 512:(n + 1) * 512],
                                          pk[:])

            def qproj(qb):
                pq = aps.tile([128, 512], F32, name=f"pq{qb}", tag="y", bufs=2)
                for kc in range(NKC):
                    nc.tensor.matmul(pq[:], wq_sb[:, kc * 128:(kc + 1) * 128],
                                     x_b[qb][:, kc * 512:(kc + 1) * 512],
                                     start=(kc == 0), stop=(kc == NKC - 1))
                nc.scalar.copy(qT_sb[:, qb * 512:(qb + 1) * 512], pq[:])

            def vproj(cb):
                pv = aps.tile([128, 128], F32, name=f"pv{cb}", tag="y", bufs=2)
                n, sub = cb // 4, cb % 4
                for kc in range(NKC):
                    c0 = kc * 512 + sub * 128
                    nc.tensor.matmul(pv[:], ctx_b[n][:, c0:c0 + 128],
                                     wv_sb[:, kc * 128:(kc + 1) * 128],
                                     start=(kc == 0), stop=(kc == NKC - 1))
                if cb % 2 == 0:
                    nc.scalar.copy(v4[:, :, cb, 0:DH],
                                   pv.rearrange("p (h m) -> p h m", m=DH))
                else:
                    nc.vector.tensor_copy(
                        v4[:, :, cb, 0:DH],
                        pv.rearrange("p (h m) -> p h m", m=DH))

            # ---- ramp: all projections, paced by the n-major input DMA
            for n in range(NQB):
                kproj(n)
                if n == 0:
                    qproj(0)
                for cb in range(4 * n, 4 * n + 4):
                    vproj(cb)

            # ---- epilogue pieces (for the PREVIOUS qb), spread over the
            # first groups of the next qb's h=0 pass:
            #   g0: recip + normalize qs 0,1 (+XBAR transposes)
            #   g1: normalize qs 2,3
            #   g4+qs: out-proj matmul; g5+qs: y copy + DMA out
            class Epilogue:
                def __init__(self, o_ps_p, qbp):
                    self.o_ps = o_ps_p
                    self.qb = qbp
                    self.oTs = []
                    self.pys = {}

                def norm(self, final=False):
                    # reciprocal of the 8 denominators into SBUF, then ONE
                    # broadcast multiply packs o/den into [128, 512] bf16
                    # (region r of o2n = (qs, h) in offset order, so the
                    # transpose slices match the old per-qs t-tile layout).
                    # Both on DVE back-to-back; the engine can read only one
                    # PSUM input per instruction, so rc must come from SBUF.
                    ov = self.o_ps.rearrange("p (r w) -> p r w", w=128)
                    rc = msb.tile([128, 8], F32, name=f"rc{self.qb}",
                                  tag="rc", bufs=2)
                    nc.vector.reciprocal(rc[:].unsqueeze(2),
                                         ov[:, :, DH:DH + 1])
                    o2n = msb.tile([128, 512], BF16, name=f"o2n{self.qb}",
                                   tag="o2n", bufs=2)
                    rcb = rc[:].unsqueeze(2).broadcast_to([128, 8, DH])
                    nc.vector.tensor_tensor(
                        o2n.rearrange("p (r w) -> p r w", w=DH),
                        ov[:, :, 0:DH], rcb, MULT)
                    self.o2n = o2n
                    if final:
                        return  # finish() transposes on the PE instead
                    # one batched XBAR transpose: out[:, t, :] = tile t of
                    # o2n transposed (3D out => per-128-tile transpose)
                    self.oTs = msb.tile([128, 512], BF16,
                                        name=f"oT{self.qb}", tag="oT",
                                        bufs=2)
                    nc.sync.dma_start_transpose(
                        self.oTs.rearrange("p (t j) -> p t j", j=128),
                        o2n[:])

                def py_mm(self, qs):
                    py = aps.tile([128, 512], F32, name=f"py{self.qb}_{qs}",
                                  tag="y", bufs=2)
                    nc.tensor.matmul(py[:],
                                     self.oTs[:, qs * 128:(qs + 1) * 128],
                                     wo_sb[:], start=True, stop=True)
                    self.pys[qs] = py

                def ycopy(self, qs, eng=None):
                    ysb = msb.tile([128, 512], BF16, name=f"y{self.qb}_{qs}",
                                   tag="ysb", bufs=4)
                    nc.scalar.copy(ysb[:], self.pys[qs][:])
                    r0 = self.qb * 512 + qs * 128
                    nc.sync.dma_start(y[r0:r0 + 128, :], ysb[:])

                YC = {6: 0, 7: 1, 8: 2, 9: 3}

                def step(self, g):
                    if g == 5:
                        self.py_mm(0)
                        self.py_mm(1)
                    elif g == 6:
                        self.py_mm(2)
                        self.py_mm(3)
                    if g in self.YC:
                        self.ycopy(self.YC[g])

                def finish(self):
                    # final-qb epilogue (norm already issued at qb end): PE
                    # transposes via identity avoid the ~2.2us XBAR DMA
                    # latency, and the copies alternate ACT/DVE so the
                    # per-qs chains overlap
                    for qs in range(4):
                        pT = aps.tile([128, 128], BF16, name=f"pT{qs}",
                                      tag="sa", bufs=2)
                        nc.tensor.transpose(
                            pT[:], self.o2n[:, qs * 128:(qs + 1) * 128],
                            ident_sb[:])
                        ot = msb.tile([128, 128], BF16, name=f"otf{qs}",
                                      tag="otf", bufs=2)
                        if qs % 2 == 0:
                            nc.scalar.copy(ot[:], pT[:])
                        else:
                            nc.vector.tensor_copy(ot[:], pT[:])
                        py = aps.tile([128, 512], F32, name=f"pyf{qs}",
                                      tag="y", bufs=2)
                        nc.tensor.matmul(py[:], ot[:], wo_sb[:],
                                         start=True, stop=True)
                        ysb = msb.tile([128, 512], BF16, name=f"yf{qs}",
                                       tag="ysb", bufs=4)
                        if qs % 2 == 0:
                            nc.vector.tensor_copy(ysb[:], py[:])
                        else:
                            nc.scalar.copy(ysb[:], py[:])
                        r0 = self.qb * 512 + qs * 128
                        nc.sync.dma_start(y[r0:r0 + 128, :], ysb[:])

            # ---- attention main loop ----
            prev_ep = None
            for qb in range(NQB):
                qsl = slice(qb * 512, (qb + 1) * 512)
                o_ps = aps.tile([128, 1024], F32, name=f"o{qb}", tag="o",
                                bufs=1)
                pend = []

                def flush_pv(p_sb, h, g):
                    # start=True clears the ENTIRE psum bank's has_written
                    # bits, so only the first matmul into each bank (qs 0/2,
                    # h==0, g==0, i==0) may carry it; every other region's
                    # first write then overwrites via per-element has_written.
                    for i, cb in ((0, 2 * g), (1, 2 * g + 1)):
                        vsl = slice(h * VROW + cb * VW,
                                    h * VROW + (cb + 1) * VW)
                        for qs in range(4):
                            off = o_off(qs, h)
                            nc.tensor.matmul(
                                o_ps[:, off:off + VW],
                                p_sb[i][:, qs * 128:(qs + 1) * 128],
                                v_sb[:, vsl],
                                start=(h == 0 and g == 0 and i == 0
                                       and qs % 2 == 0),
                                stop=(h == 1 and g == NG - 1 and i == 1),
                                skip_group_check=True)

                for h in range(2):
                    hsl = slice(h * 64, (h + 1) * 64)
                    for g in range(NG):
                        # separate score tiles per exp half: each tile has
                        # exactly one reader, so the tile framework never
                        # chains the ACT and DVE exps behind one another
                        # (multi-reader tiles get their readers serialized
                        # to keep downstream WAR waits single-sem).
                        sa_ps = aps.tile([128, 512], F32,
                                         name=f"sa{qb}_{h}_{g}", tag="sa",
                                         bufs=2)
                        sb_ps = aps.tile([128, 512], F32,
                                         name=f"sb{qb}_{h}_{g}", tag="sb",
                                         bufs=2)
                        for i, cb in ((0, 2 * g), (1, 2 * g + 1)):
                            csl = slice(cb * 128, (cb + 1) * 128)
                            nc.tensor.matmul((sa_ps, sb_ps)[i][:],
                                             kT_sb[hsl, csl],
                                             qT_sb[hsl, qsl],
                                             start=True, stop=True)
                        pa_sb = psb.tile([128, 512], BF16,
                                         name=f"pa{qb}_{h}_{g}", tag="pa",
                                         bufs=6)
                        pb_sb = psb.tile([128, 512], BF16,
                                         name=f"pb{qb}_{h}_{g}", tag="pb",
                                         bufs=6)
                        nc.vector.tensor_scalar(
                            pa_sb[:].bitcast(I16), sa_ps[:],
                            SCH_A, SCH_B, MULT, ADD)
                        nc.scalar.activation(pb_sb[:], sb_ps[:],
                                             EXP, scale=SCALE)
                        p_sb = (pa_sb, pb_sb)
                        pend.append((p_sb, h, g))
                        if len(pend) > LAG:
                            flush_pv(*pend.pop(0))
                        if h == 0 and prev_ep is not None:
                            prev_ep.step(g)
                        if h == 1 and g == QPROJ_G and qb + 1 < NQB:
                            qproj(qb + 1)
                for t in pend:
                    flush_pv(*t)
                prev_ep = Epilogue(o_ps, qb)
                prev_ep.norm(final=(qb == NQB - 1))

            # final epilogue (qb = NQB-1)
            prev_ep.finish()

    nc.compile()
    return nc


def make_in_maps(x, context, w_q, w_k, w_v, w_out):
    bf = ml_dtypes.bfloat16
    wqT = np.ascontiguousarray(w_q.T).astype(bf)    # [D, INNER]
    wkT = np.ascontiguousarray(w_k.T).astype(bf)
    wvT = np.ascontiguousarray(w_v.T).astype(bf)
    woT = np.ascontiguousarray(w_out.T).astype(bf)  # [INNER, D]
    ident = np.eye(128, dtype=bf)
    xTs = [np.ascontiguousarray(x[b].T).astype(bf) for b in range(B)]
    cTs = [np.ascontiguousarray(context[b].T).astype(bf) for b in range(B)]
    in_maps = []
    for c in range(8):
        b, hp = c // 4, c % 4
        hsl = slice(hp * 128, (hp + 1) * 128)
        in_maps.append({
            "xT": xTs[b],
            "ctxT": cTs[b],
            "wqT": np.ascontiguousarray(wqT[:, hsl]),
            "wkT": np.ascontiguousarray(wkT[:, hsl]),
            "wvT": np.ascontiguousarray(wvT[:, hsl]),
            "woT": np.ascontiguousarray(woT[hsl, :]),
            "ident": ident,
        })
    return in_maps


def kernel(x, context, w_q, w_k, w_v, w_out, b_out):
    x = np.asarray(x, dtype=np.float32)
    context = np.asarray(context, dtype=np.float32)
    w_q = np.asarray(w_q, dtype=np.float32)
    w_k = np.asarray(w_k, dtype=np.float32)
    w_v = np.asarray(w_v, dtype=np.float32)
    w_out = np.asarray(w_out, dtype=np.float32)
    b_out = np.asarray(b_out, dtype=np.float32)

    if "nc" not in _CACHE:
        _CACHE["nc"] = build_nc()
    nc = _CACHE["nc"]

    in_maps = make_in_maps(x, context, w_q, w_k, w_v, w_out)
    res = run_bass_kernel_spmd(nc, in_maps, list(range(8))).results

    out = np.zeros((B, S, D), dtype=np.float32)
    for c in range(8):
        out[c // 4] += np.asarray(res[c]["y"], dtype=np.float32)
    out += b_out
    return out


# revision 88
# speedup vs baseline: 1.0003x; 1.0003x over previous
"""Trainium2 Bass kernel for nn_CrossAttention (B=2, S=C=4096, D=512, H=8, Dh=64).

Sharding: batch x head-pair parallel over 8 cores. Core c handles batch
b = c//4 and heads {2*(c%4), 2*(c%4)+1}. Each core computes full attention
for its two heads plus its partial contribution to the output projection;
the host sums the 4 per-core partials per batch and adds the bias.

All-bf16 dataflow (inputs pre-rounded on host; fp32 PSUM accumulation):
  kT [128=2*dh, C] = wk_sb.T @ ctx_b          (N=512 moving)
  qT [128=2*dh, S] = wq_sb.T @ x_b
  v  [c, 128=2*dh] = ctx_b.T @ wv_sb          (N=128 moving, bf16)
  s chunk pair -> TWO PSUM tiles sa/sb [128c, 512q] = kT_h.T @ qT_h; each
      tile has exactly ONE exp reader (DVE does sa as Schraudolph bit-trick
      exp -- bf16_bits = int16(A*s + B) via int16 bitcast -- and ACT does sb
      as true exp; the slower DVE gets the earlier-finishing chunk so the
      per-group exp envelope is tightest).  One reader per tile keeps
      the tile framework from chaining the two exps behind each other, so
      the ACT and DVE software pipelines run fully decoupled and the
      s-pool WAR chain (bufs=2 each) is half a group shorter.
  o_aug [128q, 65] += P_chunk.T @ v_aug_chunk  (P stationary, v moving
      N=65; col 64 = ones -> softmax denominator lands per-q-partition;
      flush lags exp issue by LAG groups)
  o2n [128q, 512] = o/den via DVE reciprocal + ONE broadcast multiply
      (issued at the END of each qb so the next qb's first PV never waits)
  oT = one batched XBAR dma transpose (3D out => 4x 128x128 tiles)
  y   [128s, 512]  = oT.T @ woT (K=128, both heads at once) -> bf16 y
      partials (host accumulates in f32)

Engine budget per group (~745ns of PE matmul): ACT 612ns exp + the
projection/y copies, DVE 658ns exp + the normalize; all PSUM-reading work
must stay on ACT/DVE (GPSIMD cannot touch PSUM).  The final qb's epilogue
uses PE transposes via an identity matrix (no XBAR sem latency) with
copies alternating ACT/DVE.  Inputs load as ONE packed DMA per 512-col
block spanning all 4 contraction chunks (DMA cost is dominated by per-
instruction generation, not bytes), n-major so kproj/vproj pace with
arrival; the v ones-columns are memset (a strided DMA of ones costs
~3.6us of descriptor time).
"""

import numpy as np
import ml_dtypes
from contextlib import ExitStack

import concourse.bass as bass
import concourse.tile as tile
from concourse import bacc, mybir
from concourse.bass_utils import run_bass_kernel_spmd

F32 = mybir.dt.float32
BF16 = mybir.dt.bfloat16
I16 = mybir.dt.int16
EXP = mybir.ActivationFunctionType.Exp
MULT = mybir.AluOpType.mult
ADD = mybir.AluOpType.add

B = 2
S = 4096
C = 4096
D = 512
DH = 64
SCALE = DH ** -0.5  # 0.125
NKC = D // 128      # 4 contraction chunks
NQB = S // 512      # 8 query blocks
NCB = C // 128      # 32 context chunks of 128
NG = NCB // 2       # 16 chunk groups of 2 per (h, qb)
VW = DH + 1         # 65
VROW = NCB * VW     # per-head width of the v_aug tile

LAG = 3             # PV flush lag (groups); keeps PE off the o_ps WAR
QPROJ_G = 14         # issue qproj(qb+1) at (h=1, g=QPROJ_G)

# Schraudolph exp in bf16-bits domain: bits = int16(A*s + B)
SCH_A = SCALE * 128.0 / float(np.log(2.0))
SCH_B = 16256.0 - 5.25

_CACHE = {}


def build_nc():
    nc = bacc.Bacc("TRN2", target_bir_lowering=False, debug=False)

    xT = nc.dram_tensor("xT", [D, S], BF16, kind="ExternalInput").ap()
    ctxT = nc.dram_tensor("ctxT", [D, C], BF16, kind="ExternalInput").ap()
    wqT = nc.dram_tensor("wqT", [D, 128], BF16, kind="ExternalInput").ap()
    wkT = nc.dram_tensor("wkT", [D, 128], BF16, kind="ExternalInput").ap()
    wvT = nc.dram_tensor("wvT", [D, 128], BF16, kind="ExternalInput").ap()
    woT = nc.dram_tensor("woT", [128, D], BF16, kind="ExternalInput").ap()
    ident = nc.dram_tensor("ident", [128, 128], BF16,
                           kind="ExternalInput").ap()
    y = nc.dram_tensor("y", [S, D], BF16, kind="ExternalOutput").ap()

    def o_off(qs, h):
        # o_aug slice offsets inside the [128, 1024] accumulator; regions
        # are padded to a uniform 128-word stride (so the 8 denominators at
        # +64 form one strided AP) and stay inside their 512-word bank.
        return (qs // 2) * 512 + ((qs % 2) * 2 + h) * 128

    with tile.TileContext(nc) as tc, ExitStack() as ctx:
        sb = ctx.enter_context(tc.tile_pool(name="sb", bufs=1))

        # ---- persistent SBUF tiles ----
        wq_sb = sb.tile([128, D], BF16, name="wq_sb")
        wk_sb = sb.tile([128, D], BF16, name="wk_sb")
        wv_sb = sb.tile([128, D], BF16, name="wv_sb")
        wo_sb = sb.tile([128, D], BF16, name="wo_sb")
        kT_sb = sb.tile([128, C], BF16, name="kT_sb")
        qT_sb = sb.tile([128, S], BF16, name="qT_sb")
        v_sb = sb.tile([128, 2 * VROW], BF16, name="v_sb")
        ident_sb = sb.tile([128, 128], BF16, name="ident_sb")

        with tc.tile_pool(name="aps", bufs=1, space="PSUM") as aps, \
             tc.tile_pool(name="inbig", bufs=8) as inbig, \
             tc.tile_pool(name="psb", bufs=6) as psb, \
             tc.tile_pool(name="msb", bufs=2) as msb:
            ctx_b = [inbig.tile([128, NKC * 512], BF16, name=f"ctxb{n}",
                                tag="in", bufs=16) for n in range(NQB)]
            x_b = [inbig.tile([128, NKC * 512], BF16, name=f"xb{n}",
                              tag="in", bufs=16) for n in range(NQB)]

            # ---- DMA order tuned for ramp: k/q weights, block 0 of both
            # inputs, v/o weights, then the remaining blocks n-major so
            # kproj(n)/vproj(4n..4n+3) can start as block n lands.
            def dma_w(dst, src):
                nc.sync.dma_start(
                    dst.rearrange("p (kc m) -> p kc m", m=128),
                    src.rearrange("(kc p) m -> p kc m", p=128))

            def load_block(dst, srcT, n):
                # one DMA per 512-col block spanning all NKC row-chunks:
                # generation cost is per-DMA, so packed blocks keep the DMA
                # pipeline transfer-bound instead of generation-bound
                nc.sync.dma_start(
                    dst.rearrange("p (kc m) -> p kc m", m=512),
                    srcT.rearrange("(kc p) m -> p kc m",
                                   p=128)[:, :, n * 512:(n + 1) * 512])

            def load_half(dst, srcT, n, half):
                k0 = half * 2
                nc.sync.dma_start(
                    dst[:, k0 * 512:(k0 + 2) * 512].rearrange(
                        "p (kc m) -> p kc m", m=512),
                    srcT.rearrange("(kc p) m -> p kc m",
                                   p=128)[:, k0:k0 + 2,
                                          n * 512:(n + 1) * 512])

            dma_w(wk_sb, wkT)
            load_half(ctx_b[0], ctxT, 0, 0)
            dma_w(wq_sb, wqT)
            load_half(x_b[0], xT, 0, 0)
            load_half(ctx_b[0], ctxT, 0, 1)
            load_half(x_b[0], xT, 0, 1)
            dma_w(wv_sb, wvT)
            # ones columns of v_aug via memset (a strided DMA of ones costs
            # ~3.6us of descriptor time and blocks the ctx loads)
            v4 = v_sb.rearrange("p (h c k) -> p h c k", h=2, k=VW)
            nc.gpsimd.memset(v4[:, :, :, DH:VW], 1.0)
            for n in range(1, NQB):
                load_block(ctx_b[n], ctxT, n)
            nc.sync.dma_start(wo_sb[:], woT)
            nc.sync.dma_start(ident_sb[:], ident)
            for n in range(1, NQB):
                load_block(x_b[n], xT, n)

            # warm the PE p-state: the clock model needs ~3us of
            # continuous busy to reach 2.4GHz, and the first projections
            # otherwise run at 0.65-1.2GHz while waiting on input DMA
            dum_sb = sb.tile([128, 128], BF16, name="dum_sb")
            nc.gpsimd.memset(dum_sb[:], 0.0)
            dum_ps = aps.tile([128, 128], F32, name="dum_ps", tag="y",
                              bufs=2)
            for _ in range(16):
                nc.tensor.matmul(dum_ps[:], dum_sb[:], dum_sb[:],
                                 start=True, stop=True)

            def kproj(n):
                pk = aps.tile([128, 512], F32, name=f"pk{n}", tag="y", bufs=2)
                for kc in range(NKC):
                    nc.tensor.matmul(pk[:], wk_sb[:, kc * 128:(kc + 1) * 128],
                                     ctx_b[n][:, kc * 512:(kc + 1) * 512],
                                     start=(kc == 0), stop=(kc == NKC - 1))
                if n % 2 == 0:
                    nc.scalar.copy(kT_sb[:, n * 512:(n + 1) * 512], pk[:])
                else:
                    nc.vector.tensor_copy(kT_sb[:, n * 512:(n + 1) * 512],
                                          pk[:])

            def qproj(qb):
                pq = aps.tile([128, 512], F32, name=f"pq{qb}", tag="y", bufs=2)
                for kc in range(NKC):
                    nc.tensor.matmul(pq[:], wq_sb[:, kc * 128:(kc + 1) * 128],
                                     x_b[qb][:, kc * 512:(kc + 1) * 512],
                                     start=(kc == 0), stop=(kc == NKC - 1))
                nc.scalar.copy(qT_sb[:, qb * 512:(qb + 1) * 512], pq[:])

            def vproj(cb):
                pv = aps.tile([128, 128], F32, name=f"pv{cb}", tag="y", bufs=2)
                n, sub = cb // 4, cb % 4
                for kc in range(NKC):
                    c0 = kc * 512 + sub * 128
                    nc.tensor.matmul(pv[:], ctx_b[n][:, c0:c0 + 128],
                                     wv_sb[:, kc * 128:(kc + 1) * 128],
                                     start=(kc == 0), stop=(kc == NKC - 1))
                if cb % 2 == 0:
                    nc.scalar.copy(v4[:, :, cb, 0:DH],
                                   pv.rearrange("p (h m) -> p h m", m=DH))
                else:
                    nc.vector.tensor_copy(
                        v4[:, :, cb, 0:DH],
                        pv.rearrange("p (h m) -> p h m", m=DH))

            # ---- ramp: all projections, paced by the n-major input DMA
            for n in range(NQB):
                kproj(n)
                if n == 0:
                    qproj(0)
                for cb in range(4 * n, 4 * n + 4):
                    vproj(cb)

            # ---- epilogue pieces (for the PREVIOUS qb), spread over the
            # first groups of the next qb's h=0 pass:
            #   g0: recip + normalize qs 0,1 (+XBAR transposes)
            #   g1: normalize qs 2,3
            #   g4+qs: out-proj matmul; g5+qs: y copy + DMA out
            class Epilogue:
                def __init__(self, o_ps_p, qbp):
                    self.o_ps = o_ps_p
                    self.qb = qbp
                    self.oTs = []
                    self.pys = {}

                def norm(self, final=False):
                    # reciprocal of the 8 denominators into SBUF, then ONE
                    # broadcast multiply packs o/den into [128, 512] bf16
                    # (region r of o2n = (qs, h) in offset order, so the
                    # transpose slices match the old per-qs t-tile layout).
                    # Both on DVE back-to-back; the engine can read only one
                    # PSUM input per instruction, so rc must come from SBUF.
                    ov = self.o_ps.rearrange("p (r w) -> p r w", w=128)
                    rc = msb.tile([128, 8], F32, name=f"rc{self.qb}",
                                  tag="rc", bufs=2)
                    nc.vector.reciprocal(rc[:].unsqueeze(2),
                                         ov[:, :, DH:DH + 1])
                    o2n = msb.tile([128, 512], BF16, name=f"o2n{self.qb}",
                                   tag="o2n", bufs=2)
                    rcb = rc[:].unsqueeze(2).broadcast_to([128, 8, DH])
                    nc.vector.tensor_tensor(
                        o2n.rearrange("p (r w) -> p r w", w=DH),
                        ov[:, :, 0:DH], rcb, MULT)
                    self.o2n = o2n
                    if final:
                        return  # finish() transposes on the PE instead
                    # one batched XBAR transpose: out[:, t, :] = tile t of
                    # o2n transposed (3D out => per-128-tile transpose)
                    self.oTs = msb.tile([128, 512], BF16,
                                        name=f"oT{self.qb}", tag="oT",
                                        bufs=2)
                    nc.sync.dma_start_transpose(
                        self.oTs.rearrange("p (t j) -> p t j", j=128),
                        o2n[:])

                def py_mm(self, qs):
                    py = aps.tile([128, 512], F32, name=f"py{self.qb}_{qs}",
                                  tag="y", bufs=2)
                    nc.tensor.matmul(py[:],
                                     self.oTs[:, qs * 128:(qs + 1) * 128],
                                     wo_sb[:], start=True, stop=True)
                    self.pys[qs] = py

                def ycopy(self, qs, eng=None):
                    ysb = msb.tile([128, 512], BF16, name=f"y{self.qb}_{qs}",
                                   tag="ysb", bufs=4)
                    nc.scalar.copy(ysb[:], self.pys[qs][:])
                    r0 = self.qb * 512 + qs * 128
                    nc.sync.dma_start(y[r0:r0 + 128, :], ysb[:])

                YC = {5: 0, 6: 1, 7: 2, 8: 3}

                def step(self, g):
                    if g == 5:
                        self.py_mm(0)
                        self.py_mm(1)
                    elif g == 6:
                        self.py_mm(2)
                        self.py_mm(3)
                    if g in self.YC:
                        self.ycopy(self.YC[g])

                def finish(self):
                    # final-qb epilogue (norm already issued at qb end): PE
                    # transposes via identity avoid the ~2.2us XBAR DMA
                    # latency, and the copies alternate ACT/DVE so the
                    # per-qs chains overlap
                    for qs in range(4):
                        pT = aps.tile([128, 128], BF16, name=f"pT{qs}",
                                      tag="sa", bufs=2)
                        nc.tensor.transpose(
                            pT[:], self.o2n[:, qs * 128:(qs + 1) * 128],
                            ident_sb[:])
                        ot = msb.tile([128, 128], BF16, name=f"otf{qs}",
                                      tag="otf", bufs=2)
                        if qs % 2 == 0:
                            nc.scalar.copy(ot[:], pT[:])
                        else:
                            nc.vector.tensor_copy(ot[:], pT[:])
                        py = aps.tile([128, 512], F32, name=f"pyf{qs}",
                                      tag="y", bufs=2)
                        nc.tensor.matmul(py[:], ot[:], wo_sb[:],
                                         start=True, stop=True)
                        ysb = msb.tile([128, 512], BF16, name=f"yf{qs}",
                                       tag="ysb", bufs=4)
                        if qs % 2 == 0:
                            nc.vector.tensor_copy(ysb[:], py[:])
                        else:
                            nc.scalar.copy(ysb[:], py[:])
                        r0 = self.qb * 512 + qs * 128
                        nc.sync.dma_start(y[r0:r0 + 128, :], ysb[:])

            # ---- attention main loop ----
            prev_ep = None
            for qb in range(NQB):
                qsl = slice(qb * 512, (qb + 1) * 512)
                o_ps = aps.tile([128, 1024], F32, name=f"o{qb}", tag="o",
                                bufs=1)
                pend = []

                def flush_pv(p_sb, h, g):
                    # start=True clears the ENTIRE psum bank's has_written
                    # bits, so only the first matmul into each bank (qs 0/2,
                    # h==0, g==0, i==0) may carry it; every other region's
                    # first write then overwrites via per-element has_written.
                    for i, cb in ((0, 2 * g), (1, 2 * g + 1)):
                        vsl = slice(h * VROW + cb * VW,
                                    h * VROW + (cb + 1) * VW)
                        for qs in range(4):
                            off = o_off(qs, h)
                            nc.tensor.matmul(
                                o_ps[:, off:off + VW],
                                p_sb[i][:, qs * 128:(qs + 1) * 128],
                                v_sb[:, vsl],
                                start=(h == 0 and g == 0 and i == 0
                                       and qs % 2 == 0),
                                stop=(h == 1 and g == NG - 1 and i == 1),
                                skip_group_check=True)

                for h in range(2):
                    hsl = slice(h * 64, (h + 1) * 64)
                    for g in range(NG):
                        # separate score tiles per exp half: each tile has
                        # exactly one reader, so the tile framework never
                        # chains the ACT and DVE exps behind one another
                        # (multi-reader tiles get their readers serialized
                        # to keep downstream WAR waits single-sem).
                        sa_ps = aps.tile([128, 512], F32,
                                         name=f"sa{qb}_{h}_{g}", tag="sa",
                                         bufs=2)
                        sb_ps = aps.tile([128, 512], F32,
                                         name=f"sb{qb}_{h}_{g}", tag="sb",
                                         bufs=2)
                        for i, cb in ((0, 2 * g), (1, 2 * g + 1)):
                            csl = slice(cb * 128, (cb + 1) * 128)
                            nc.tensor.matmul((sa_ps, sb_ps)[i][:],
                                             kT_sb[hsl, csl],
                                             qT_sb[hsl, qsl],
                                             start=True, stop=True)
                        pa_sb = psb.tile([128, 512], BF16,
                                         name=f"pa{qb}_{h}_{g}", tag="pa",
                                         bufs=6)
                        pb_sb = psb.tile([128, 512], BF16,
                                         name=f"pb{qb}_{h}_{g}", tag="pb",
                                         bufs=6)
                        nc.vector.tensor_scalar(
                            pa_sb[:].bitcast(I16), sa_ps[:],
                            SCH_A, SCH_B, MULT, ADD)
                        nc.scalar.activation(pb_sb[:], sb_ps[:],
                                             EXP, scale=SCALE)
                        p_sb = (pa_sb, pb_sb)
                        pend.append((p_sb, h, g))
                        if len(pend) > LAG:
                            flush_pv(*pend.pop(0))
                        if h == 0 and prev_ep is not None:
                            prev_ep.step(g)
                        if h == 1 and g == QPROJ_G and qb + 1 < NQB:
                            qproj(qb + 1)
                for t in pend:
                    flush_pv(*t)
                prev_ep = Epilogue(o_ps, qb)
                prev_ep.norm(final=(qb == NQB - 1))

            # final epilogue (qb = NQB-1)
            prev_ep.finish()

    nc.compile()
    return nc


def make_in_maps(x, context, w_q, w_k, w_v, w_out):
    bf = ml_dtypes.bfloat16
    wqT = np.ascontiguousarray(w_q.T).astype(bf)    # [D, INNER]
    wkT = np.ascontiguousarray(w_k.T).astype(bf)
    wvT = np.ascontiguousarray(w_v.T).astype(bf)
    woT = np.ascontiguousarray(w_out.T).astype(bf)  # [INNER, D]
    ident = np.eye(128, dtype=bf)
    xTs = [np.ascontiguousarray(x[b].T).astype(bf) for b in range(B)]
    cTs = [np.ascontiguousarray(context[b].T).astype(bf) for b in range(B)]
    in_maps = []
    for c in range(8):
        b, hp = c // 4, c % 4
        hsl = slice(hp * 128, (hp + 1) * 128)
        in_maps.append({
            "xT": xTs[b],
            "ctxT": cTs[b],
            "wqT": np.ascontiguousarray(wqT[:, hsl]),
            "wkT": np.ascontiguousarray(wkT[:, hsl]),
            "wvT": np.ascontiguousarray(wvT[:, hsl]),
            "woT": np.ascontiguousarray(woT[hsl, :]),
            "ident": ident,
        })
    return in_maps


def kernel(x, context, w_q, w_k, w_v, w_out, b_out):
    x = np.asarray(x, dtype=np.float32)
    context = np.asarray(context, dtype=np.float32)
    w_q = np.asarray(w_q, dtype=np.float32)
    w_k = np.asarray(w_k, dtype=np.float32)
    w_v = np.asarray(w_v, dtype=np.float32)
    w_out = np.asarray(w_out, dtype=np.float32)
    b_out = np.asarray(b_out, dtype=np.float32)

    if "nc" not in _CACHE:
        _CACHE["nc"] = build_nc()
    nc = _CACHE["nc"]

    in_maps = make_in_maps(x, context, w_q, w_k, w_v, w_out)
    res = run_bass_kernel_spmd(nc, in_maps, list(range(8))).results

    out = np.zeros((B, S, D), dtype=np.float32)
    for c in range(8):
        out[c // 4] += np.asarray(res[c]["y"], dtype=np.float32)
    out += b_out
    return out
